# revision 6
# baseline (speedup 1.0000x reference)
"""Causal single-head attention on 8 trn2 NeuronCores.

Problem: x [4, 2048, 1024] f32; Wq/Wk/Wv [1024, 1024] f32.
  q,k,v = x@W*; scores = q@k^T (causal masked, scaled 1/sqrt(1024));
  out = softmax(scores) @ v.

Sharding: 8 cores = 4 batches x 2 query-halves. Core c: batch c//2,
parity h=c%2 owns query cols {0,3} (h=0) or {1,2} (h=1) in units of 512
rows -- an exactly load-balanced causal split. The SPMD program is made
uniform across cores by padding causal extents to {2,4} key-cols for the
two local query cols and passing per-core causal masks as data.

Per-core kernel (all matmuls in float32r: 1 cyc/row at FD=512):
  phase 1:  KT[e,kn] = Wk-chunks^T . xT      (stationary Wk, moving xT)
            QT[e,qn] = Wq-chunks^T . xTq
  phase 2, per local query col (512 wide):
    scoresT[kn,qn] = KT-chunks^T . QT        (PSUM, 8 e-block accum)
    expT = exp(scoresT/32)  (ACT, no max-subtraction needed: |s|/32 < ~3)
    masked tiles *= mask    (host-provided causal masks, DVE)
    rowsum[qn,1] = expT-chunks^T . ones      (tiny N=1 matmuls)
    TT[d,qn]     = x-chunks^T . expT         (x natural layout; V never built)
    out[qn,e]    = TT-chunks^T . Wv          (assoc.: (A@x)@Wv == A@(x@Wv))
    out *= 1/rowsum (per-partition scalar), DMA out.

kernel() is self-contained: shards on host, runs via run_bass_kernel_spmd
on cores 0-7, reassembles the full [4, 2048, 1024] output.
"""

import numpy as np
from contextlib import ExitStack

import concourse.bass as bass
import concourse.mybir as mybir
import concourse.tile as tile
from concourse import bacc
from concourse.bass_utils import run_bass_kernel_spmd

P = 128
D = 1024          # d_in == d_out
NSEQ = 2048
NCOL = 512        # moving free dim / column unit
DB = D // P       # 8 d blocks
EB = D // P       # 8 e blocks
EXT = (2, 4)      # padded causal extent (in 512-cols of keys) per local q col
QCOLS = {0: (0, 3), 1: (1, 2)}  # parity -> global q cols (512 units)

_f32 = mybir.dt.float32
_f32r = mybir.dt.float32r

_BUILD_CACHE = {}


def _build():
    if "nc" in _BUILD_CACHE:
        return _BUILD_CACHE["nc"]

    nc = bacc.Bacc("TRN2", target_bir_lowering=False, debug=False, num_devices=8)
    xt = nc.dram_tensor("xt", [D, NSEQ], _f32, kind="ExternalInput").ap()
    xtq = nc.dram_tensor("xtq", [D, 1024], _f32, kind="ExternalInput").ap()
    xk = nc.dram_tensor("xk", [NSEQ, D], _f32, kind="ExternalInput").ap()
    wq = nc.dram_tensor("wq", [D, D], _f32, kind="ExternalInput").ap()
    wk = nc.dram_tensor("wk", [D, D], _f32, kind="ExternalInput").ap()
    wv = nc.dram_tensor("wv", [D, D], _f32, kind="ExternalInput").ap()
    msk = nc.dram_tensor("msk", [16, P, NCOL], _f32, kind="ExternalInput").ap()
    onesd = nc.dram_tensor("ones", [P, 1], _f32, kind="ExternalInput").ap()
    out = nc.dram_tensor("out", [1024, D], _f32, kind="ExternalOutput").ap()

    scale = float(1.0 / np.sqrt(D))

    with tile.TileContext(nc) as tc, ExitStack() as ctx:
        pers = ctx.enter_context(tc.tile_pool(name="pers", bufs=1))
        KT = pers.tile([P, EB, 4, NCOL], _f32r)      # 64 KB/part
        QT = pers.tile([P, EB, 2, NCOL], _f32r)      # 32
        RCPB = pers.tile([P, NCOL], _f32)            # 1/rowsum bcast (shared)
        ONES = pers.tile([P, 1], _f32r)
        nc.sync.dma_start(ONES[:], onesd.bitcast(_f32r))

        # ---- phase 1: KT / QT projections ----
        with ExitStack() as p1:
            wpool = p1.enter_context(tc.tile_pool(name="wpool", bufs=1))
            WK = wpool.tile([P, DB, EB, P], _f32r)   # 32
            WQ = wpool.tile([P, DB, EB, P], _f32r)   # 32
            xcol = p1.enter_context(tc.tile_pool(name="xcol", bufs=2))
            ps_proj = p1.enter_context(tc.tile_pool(name="ps_proj", bufs=4, space="PSUM"))

            nc.sync.dma_start(
                WK[:], wk.rearrange("(db p) (eb m) -> p db eb m", p=P, m=P).bitcast(_f32r))
            nc.sync.dma_start(
                WQ[:], wq.rearrange("(db p) (eb m) -> p db eb m", p=P, m=P).bitcast(_f32r))

            for ic in range(4):
                xc = xcol.tile([P, DB, NCOL], _f32r, tag="xc")
                nc.sync.dma_start(
                    xc[:],
                    xt[:, ic * NCOL:(ic + 1) * NCOL]
                    .rearrange("(db p) n -> p db n", p=P).bitcast(_f32r))
                for eb in range(EB):
                    ps = ps_proj.tile([P, NCOL], _f32)
                    for db in range(DB):
                        nc.tensor.matmul(ps[:], WK[:, db, eb, :], xc[:, db, :],
                                         start=(db == 0), stop=(db == DB - 1))
                    nc.vector.tensor_copy(KT[:, eb, ic, :], ps[:].bitcast(_f32r))
            for jc in range(2):
                xc = xcol.tile([P, DB, NCOL], _f32r, tag="xc")
                nc.sync.dma_start(
                    xc[:],
                    xtq[:, jc * NCOL:(jc + 1) * NCOL]
                    .rearrange("(db p) n -> p db n", p=P).bitcast(_f32r))
                for eb in range(EB):
                    ps = ps_proj.tile([P, NCOL], _f32)
                    for db in range(DB):
                        nc.tensor.matmul(ps[:], WQ[:, db, eb, :], xc[:, db, :],
                                         start=(db == 0), stop=(db == DB - 1))
                    nc.vector.tensor_copy(QT[:, eb, jc, :], ps[:].bitcast(_f32r))

        # ---- phase 2: attention ----
        with ExitStack() as p2:
            p2sb = p2.enter_context(tc.tile_pool(name="p2sb", bufs=1))
            WV = p2sb.tile([P, DB, 2, NCOL], _f32r)      # 32
            EXPS = p2sb.tile([P, 16, NCOL], _f32r)       # 32
            TT = p2sb.tile([P, DB, NCOL], _f32r)         # 16
            nc.sync.dma_start(
                WV[:], wv.rearrange("(db p) (ec n) -> p db ec n", p=P, n=NCOL).bitcast(_f32r))
            ps_sc = p2.enter_context(tc.tile_pool(name="ps_sc", bufs=2, space="PSUM"))
            ps_rs = p2.enter_context(tc.tile_pool(name="ps_rs", bufs=2, space="PSUM"))
            ps_tt = p2.enter_context(tc.tile_pool(name="ps_tt", bufs=2, space="PSUM"))
            ps_out = p2.enter_context(tc.tile_pool(name="ps_out", bufs=2, space="PSUM"))
            mpool = p2.enter_context(tc.tile_pool(name="mpool", bufs=2))
            spool = p2.enter_context(tc.tile_pool(name="spool", bufs=1))
            xkpool = p2.enter_context(tc.tile_pool(name="xkpool", bufs=2))
            opool = p2.enter_context(tc.tile_pool(name="opool", bufs=2))

            for jc in range(2):
                K = 4 * EXT[jc]  # kn blocks this col
                # scores + exp (+ causal mask where needed)
                for kb in range(K):
                    ps = ps_sc.tile([P, NCOL], _f32)
                    ic, off = kb // 4, (kb % 4) * P
                    for eb in range(EB):
                        nc.tensor.matmul(ps[:], KT[:, eb, ic, off:off + P],
                                         QT[:, eb, jc, :],
                                         start=(eb == 0), stop=(eb == EB - 1))
                    nc.scalar.activation(EXPS[:, kb, :], ps[:],
                                         mybir.ActivationFunctionType.Exp,
                                         scale=scale)
                    if jc == 0 or kb >= 8:
                        mt = mpool.tile([P, NCOL], _f32r, tag="mt")
                        nc.sync.dma_start(mt[:], msk[kb, :, :].bitcast(_f32r))
                        nc.vector.tensor_mul(EXPS[:, kb, :], EXPS[:, kb, :], mt[:])
                # rowsums: ones^T @ expT -> [1, qn]; recip; bcast to all parts
                rs = ps_rs.tile([1, NCOL], _f32)
                for kb in range(K):
                    nc.tensor.matmul(rs[0:1, :], ONES[:], EXPS[:, kb, :],
                                     start=(kb == 0), stop=(kb == K - 1))
                rcp1 = spool.tile([1, NCOL], _f32, tag="rcp1")
                nc.vector.reciprocal(rcp1[0:1, :], rs[0:1, :])
                nc.gpsimd.partition_broadcast(RCPB[:, :], rcp1[0:1, :])
                # TT[d, qn] = sum_kn x[kn, d] * expT[kn, qn]
                for db in range(DB):
                    xks = xkpool.tile([P, 16, P], _f32r, tag="xks")
                    nc.sync.dma_start(
                        xks[:, :K, :],
                        xk[0:K * P, db * P:(db + 1) * P]
                        .rearrange("(kb p) m -> p kb m", p=P).bitcast(_f32r))
                    pst = ps_tt.tile([P, NCOL], _f32)
                    for kb in range(K):
                        nc.tensor.matmul(pst[:], xks[:, kb, :], EXPS[:, kb, :],
                                         start=(kb == 0), stop=(kb == K - 1))
                    # fold softmax normalization into TT (commutes with @Wv)
                    nc.vector.tensor_mul(TT[:, db, :], pst[:].bitcast(_f32r),
                                         RCPB[:, :].bitcast(_f32r))
                # out[qn, e] = sum_d TT[d, qn] * Wv[d, e]; normalize; store
                for qb in range(4):
                    for ec in range(2):
                        po = ps_out.tile([P, NCOL], _f32)
                        for db in range(DB):
                            nc.tensor.matmul(po[:], TT[:, db, qb * P:(qb + 1) * P],
                                             WV[:, db, ec, :],
                                             start=(db == 0), stop=(db == DB - 1))
                        ot = opool.tile([P, NCOL], _f32, tag="ot")
                        nc.vector.tensor_copy(ot[:], po[:])
                        nc.sync.dma_start(
                            out[jc * NCOL + qb * P: jc * NCOL + (qb + 1) * P,
                                ec * NCOL:(ec + 1) * NCOL],
                            ot[:])

    nc.compile()
    _BUILD_CACHE["nc"] = nc
    return nc


def _host_inputs(x, Wq, Wk, Wv):
    in_maps = []
    for c in range(8):
        b, h = c // 2, c % 2
        g0, g1 = QCOLS[h]
        xb = np.asarray(x[b], dtype=np.float32)
        xt_h = np.ascontiguousarray(xb.T)
        qrows = np.r_[g0 * NCOL:(g0 + 1) * NCOL, g1 * NCOL:(g1 + 1) * NCOL]
        xtq_h = np.ascontiguousarray(xb[qrows].T)
        p = np.arange(P)[:, None]
        f = np.arange(NCOL)[None, :]
        m = np.empty((16, P, NCOL), dtype=np.float32)
        for i in range(8):
            m[i] = ((i * P + p) <= (g0 * NCOL + f)).astype(np.float32)
        for i in range(8, 16):
            m[i] = ((i * P + p) <= (g1 * NCOL + f)).astype(np.float32)
        in_maps.append({
            "xt": xt_h, "xtq": xtq_h, "xk": xb,
            "wq": np.asarray(Wq, np.float32),
            "wk": np.asarray(Wk, np.float32),
            "wv": np.asarray(Wv, np.float32),
            "msk": m,
            "ones": np.ones((P, 1), np.float32),
        })
    return in_maps


def kernel(x, Wq, Wk, Wv, _trace=False, _trace_kwargs=None):
    x = np.asarray(x, dtype=np.float32)
    nc = _build()
    in_maps = _host_inputs(x, Wq, Wk, Wv)
    kw = {}
    if _trace:
        kw = {"trace": True, **(_trace_kwargs or {})}
    res = run_bass_kernel_spmd(nc, in_maps, core_ids=list(range(8)), **kw)
    full = np.empty((4, NSEQ, D), dtype=np.float32)
    for c in range(8):
        b, h = c // 2, c % 2
        g0, g1 = QCOLS[h]
        o = res.results[c]["out"]
        full[b, g0 * NCOL:(g0 + 1) * NCOL] = o[:NCOL]
        full[b, g1 * NCOL:(g1 + 1) * NCOL] = o[NCOL:]
    kernel._last_results = res
    return full
